# revision 10
# baseline (speedup 1.0000x reference)
"""Criss-cross attention (width=1) Trainium2 Bass kernel.

Math note: for width=1 the criss-cross module collapses to plain unmasked
softmax attention.  The diagonal of energy_H is masked to -inf, but the
"width" logit energy_W[i] equals that same diagonal value q_i.k_i, and it is
re-appended as the (n+1)-th softmax entry.  So per query i the softmax runs
over exactly {q_i.k_j : j=0..n-1}, and

    out = gamma * (V @ softmax_j(Q^T K)) + x
    Q = relu(bn1(w_q x)),  K = relu(bn2(w_k x)),  V = relu(bn3(w_v x))

Sharding: 8 cores = (4 batches) x (2 query halves).  Each core receives x
rotated so ITS query half sits in columns 0:2048 (softmax over keys is
permutation-invariant, so key order doesn't matter).  Zero cross-core
communication.

gamma handling: |gamma| is folded into w_v on the host; sign(gamma) is the
value of the vt "ones" column, so Z accumulates sign*Z and the epilogue
reciprocal directly yields gamma/Z — no separate scale op.

Per-core phases (matmuls bf16-in / f32-psum; x arrives pre-cast to bf16
from the host, so there are no device-side casts):
  warmup   junk matmuls un-throttle HAM while the first chunks DMA.
  prep     8 chunks of 512 columns: K quad-packed for tile_position QK,
           V^T blocks (+sign column for Z), Q for the 2048 resident
           queries.  kp/qp live in the st PSUM banks, each vp in its own
           ot bank (avoids same-bank accumulation-group serialization);
           relus spread over DVE / ACT / GPSIMD.
  attn     4 query superblocks (i5) x 8 quads: QK quad (4 concurrent
           32-row matmuls) -> one exp ACTIVATE per [128,2048] quad ->
           PV accumulate into 4 [128,257] PSUM banks.  Software-
           pipelined: PE runs PV of quad g-1 while ACT exponentiates g.
  epilogue reciprocal(sign*Z), scale, transpose back to [c, i]
           (DMA-transpose on the sync queue mid-kernel, PE transpose for
           the tail), residual add, store on the gpsimd queue.
"""

import numpy as np
import ml_dtypes

_B, _C, _N, _CR = 4, 256, 4096, 32
_NCORES = 8
_HALF = _N // 2  # queries per core
_EPS = 1e-5
_VTW = _C + 1    # 257: V^T columns + sign column for Z

_BUILD_CACHE: dict = {}


def _build(has_bq: bool, has_bk: bool, has_bv: bool, neg_g: bool):
    import concourse.mybir as mybir
    import concourse.tile as tile
    from concourse import bacc
    from concourse.masks import make_identity

    f32 = mybir.dt.float32
    bf16 = mybir.dt.bfloat16
    AF = mybir.ActivationFunctionType
    ALU = mybir.AluOpType

    nc = bacc.Bacc("TRN2", target_bir_lowering=False, debug=False)

    xbf_d = nc.dram_tensor("xbf", [_C, _N], bf16, kind="ExternalInput")
    xqf_d = nc.dram_tensor("xqf", [_C, _HALF], f32, kind="ExternalInput")
    wq_d = nc.dram_tensor("wqt4", [_C, 4 * _CR], bf16, kind="ExternalInput")
    wk_d = nc.dram_tensor("wkt4", [_C, 4 * _CR], bf16, kind="ExternalInput")
    wv_d = nc.dram_tensor("wvt", [_C, _C], bf16, kind="ExternalInput")
    bq_d = nc.dram_tensor("bq4", [4 * _CR, 1], f32, kind="ExternalInput") if has_bq else None
    bk_d = nc.dram_tensor("bk4", [4 * _CR, 1], f32, kind="ExternalInput") if has_bk else None
    bv_d = nc.dram_tensor("bv", [1, _C], bf16, kind="ExternalInput") if has_bv else None
    out_d = nc.dram_tensor("out", [_C, _HALF], f32, kind="ExternalOutput")

    NCH = _N // 512       # 8 x-chunks == 8 key quads
    NI5 = _HALF // 512    # 4 query superblocks

    with tile.TileContext(nc) as tc:
        with tc.tile_pool(name="pers", bufs=1) as pers, \
             tc.tile_pool(name="ps", space="PSUM", bufs=1) as psp, \
             tc.tile_pool(name="work", bufs=2) as work:
            # ---- PSUM map: banks 0-3 = two double-buffered st half-quad
            # tiles (also kp/qp in prep), banks 4-7 the 4 ot accumulators
            # (also vp in prep)
            sts = [psp.tile([128, 1024], f32, name=f"st{h}") for h in range(2)]
            ots = [psp.tile([128, 512], f32, name=f"ot{s}") for s in range(4)]

            # ---- persistent SBUF ----
            wq_sb = pers.tile([128, 8 * _CR], bf16, name="wq_sb")
            wk_sb = pers.tile([128, 8 * _CR], bf16, name="wk_sb")
            wv_sb = pers.tile([128, 2 * _C], bf16, name="wv_sb")
            nc.sync.dma_start(wk_sb[:, 0:4 * _CR], wk_d.ap()[0:128, :])
            nc.sync.dma_start(wk_sb[:, 4 * _CR:8 * _CR], wk_d.ap()[128:256, :])
            nc.gpsimd.dma_start(wq_sb[:, 0:4 * _CR], wq_d.ap()[0:128, :])
            nc.gpsimd.dma_start(wq_sb[:, 4 * _CR:8 * _CR], wq_d.ap()[128:256, :])
            nc.scalar.dma_start(wv_sb[:, 0:_C], wv_d.ap()[0:128, :])
            nc.scalar.dma_start(wv_sb[:, _C:2 * _C], wv_d.ap()[128:256, :])
            if has_bq:
                bq_sb = pers.tile([4 * _CR, 1], f32, name="bq_sb")
                nc.scalar.dma_start(bq_sb, bq_d.ap())
            if has_bk:
                bk_sb = pers.tile([4 * _CR, 1], f32, name="bk_sb")
                nc.scalar.dma_start(bk_sb, bk_d.ap())
            if has_bv:
                bv_sb = pers.tile([1, _C], bf16, name="bv_sb")
                nc.scalar.dma_start(bv_sb, bv_d.ap())
            ones_row = pers.tile([1, 128], bf16, name="ones_row")
            nc.gpsimd.memset(ones_row, 1.0)

            xq0 = pers.tile([128, _HALF], f32, name="xq0")   # residual rows 0:128
            xq1 = pers.tile([128, _HALF], f32, name="xq1")   # residual rows 128:256
            xbf0 = pers.tile([128, _N], bf16, name="xbf0")
            xbf1 = pers.tile([128, _N], bf16, name="xbf1")
            # k_pk[:, c*512 + t*128 + jj] row 32t+d = k[d, key (4c+t)*128+jj]
            k_pk = pers.tile([128, _N], bf16, name="k_pk")
            q_rep = pers.tile([128, _HALF], bf16, name="q_rep")
            vt_sb = pers.tile([128, (_N // 128) * _VTW], bf16, name="vt_sb")
            junk = pers.tile([128, 512], bf16, name="junk")
            ident = pers.tile([128, 128], f32, name="ident")

            # ---- warmup: un-throttle HAM while DMAs land ----
            nc.vector.memset(junk, 0.0)
            for w in range(12):
                nc.tensor.matmul(
                    sts[w % 2][:, (w // 2 % 2) * 512:(w // 2 % 2 + 1) * 512],
                    junk[:, 0:128], junk, start=True, stop=True)
            # Z columns of vt are constant 1.0 for the whole kernel
            for jb in range(_N // 128):
                eng = nc.vector if jb % 2 == 0 else nc.gpsimd
                eng.memset(vt_sb[:, jb * _VTW + _C:(jb + 1) * _VTW], 1.0)

            # ---- prep: 8 chunks of 512 columns ----
            for c in range(NCH):
                sl = slice(c * 512, (c + 1) * 512)
                nc.sync.dma_start(xbf0[:, sl], xbf_d.ap()[0:128, sl])
                nc.gpsimd.dma_start(xbf1[:, sl], xbf_d.ap()[128:256, sl])
                if c < 4:   # query half residual, f32
                    nc.sync.dma_start(xq0[:, sl], xqf_d.ap()[0:128, sl])
                    nc.gpsimd.dma_start(xq1[:, sl], xqf_d.ap()[128:256, sl])

                kp = ots[c % 2][:, 0:512]
                nc.tensor.matmul(kp, wk_sb[:, 0:4 * _CR], xbf0[:, sl],
                                 start=True, stop=False)
                nc.tensor.matmul(kp, wk_sb[:, 4 * _CR:8 * _CR], xbf1[:, sl],
                                 start=False, stop=True)
                if has_bk:
                    nc.vector.tensor_scalar(k_pk[:, sl], kp, bk_sb, 0.0,
                                            ALU.add, ALU.max)
                else:
                    nc.vector.tensor_scalar_max(k_pk[:, sl], kp, 0.0)

                for j, jb in enumerate(range(4 * c, 4 * c + 4)):
                    jsl = slice(jb * 128, (jb + 1) * 128)
                    vp = ots[2 + j % 2][:, 0:256]
                    nc.tensor.matmul(vp, xbf0[:, jsl], wv_sb[:, 0:_C],
                                     start=True, stop=not has_bv)
                    nc.tensor.matmul(vp, xbf1[:, jsl], wv_sb[:, _C:2 * _C],
                                     start=False, stop=not has_bv)
                    if has_bv:
                        nc.tensor.matmul(vp, ones_row, bv_sb, start=False,
                                         stop=True)
                    vsl = slice(jb * _VTW, jb * _VTW + _C)
                    # |gamma| is folded into wv on the host: pure ReLU here.
                    if j < 2:
                        nc.scalar.activation(vt_sb[:, vsl], vp, AF.Relu)
                    else:
                        nc.vector.tensor_scalar_max(vt_sb[:, vsl], vp, 0.0)

                if c < 4:
                    qp = sts[c % 2][:, 512:1024]
                    nc.tensor.matmul(qp, wq_sb[:, 0:4 * _CR], xbf0[:, sl],
                                     start=True, stop=False)
                    nc.tensor.matmul(qp, wq_sb[:, 4 * _CR:8 * _CR], xbf1[:, sl],
                                     start=False, stop=True)
                    if has_bq:
                        nc.vector.tensor_scalar(q_rep[:, sl], qp, bq_sb, 0.0,
                                                ALU.add, ALU.max)
                    elif c % 2 == 0:
                        nc.scalar.activation(q_rep[:, sl], qp, AF.Relu)
                    else:
                        nc.vector.tensor_scalar_max(q_rep[:, sl], qp, 0.0)

            make_identity(nc, ident)

            # ---- attention: flat loop over (i5, quad, half) with PV
            # lagging one half-quad globally.  The two st tiles double-
            # buffer QK against exp, so the ACT exp stream is gapless.
            e_tiles = {}

            def qk(i5, g, h):
                isl = slice(i5 * 512, (i5 + 1) * 512)
                for t2 in range(2):
                    t = 2 * h + t2
                    nc.tensor.matmul(
                        sts[h][:, t2 * 512:(t2 + 1) * 512],
                        k_pk[32 * t:32 * t + 32,
                             g * 512 + t * 128:g * 512 + (t + 1) * 128],
                        q_rep[32 * t:32 * t + 32, isl],
                        start=True, stop=True,
                        tile_position=(32 * t, 0),
                    )

            def qexp(i5, g, h):
                e = work.tile([128, 1024], bf16, name="e_sb", tag="e", bufs=3)
                nc.scalar.activation(e, sts[h], AF.Exp)
                e_tiles[(i5, g, h)] = e

            def pv(i5, g, h):
                e = e_tiles.pop((i5, g, h))
                for s in range(4):
                    for kl in range(2):
                        jb = 4 * g + 2 * h + kl
                        nc.tensor.matmul(
                            ots[s][:, 0:_VTW],
                            e[:, kl * 512 + s * 128:kl * 512 + (s + 1) * 128],
                            vt_sb[:, jb * _VTW:(jb + 1) * _VTW],
                            start=(jb == 0), stop=(jb == _N // 128 - 1),
                        )

            def epilogue(i5):
                last = i5 == NI5 - 1
                onrms = []
                for s in range(4):
                    rz = work.tile([128, 1], f32, name="rz", tag=f"rz{s}",
                                   bufs=2)
                    nc.vector.reciprocal(rz, ots[s][:, _C:_C + 1])
                    # f32 on the last block so the PE transpose (dtype-
                    # preserving) can target the f32 st PSUM slices
                    onrm = work.tile([128, _C], f32 if last else bf16,
                                     name="onrm", tag=f"onrm{last}{s}", bufs=2)
                    if neg_g:   # sign(gamma) applied here, |gamma| is in wv
                        nc.vector.tensor_scalar(onrm, ots[s][:, 0:_C], rz,
                                                -1.0, ALU.mult, ALU.mult)
                    else:
                        nc.vector.tensor_scalar_mul(onrm, ots[s][:, 0:_C], rz)
                    onrms.append(onrm)
                for s in range(4):
                    i0 = i5 * 512 + s * 128
                    for chh in range(2):
                        xq_t = xq0 if chh == 0 else xq1
                        res = work.tile([128, 128], f32, name="res", tag="res",
                                        bufs=4)
                        if last:
                            # PE is idle now and the st banks are free.
                            tp = sts[s // 2][:, (s % 2) * 512 + chh * 128:
                                             (s % 2) * 512 + (chh + 1) * 128]
                            nc.tensor.transpose(
                                tp, onrms[s][:, chh * 128:(chh + 1) * 128],
                                ident)
                            nc.vector.tensor_add(res, tp, xq_t[:, i0:i0 + 128])
                            seng = nc.sync if chh == 0 else nc.gpsimd
                        else:
                            tT = work.tile([128, 128], bf16, name="tT",
                                           tag=f"tT{chh}", bufs=4)
                            nc.sync.dma_start(
                                tT, onrms[s][:, chh * 128:(chh + 1) * 128],
                                transpose=True)
                            adde = nc.vector if chh == 0 else nc.gpsimd
                            adde.tensor_tensor(res, tT, xq_t[:, i0:i0 + 128],
                                               ALU.add)
                            seng = nc.gpsimd
                        seng.dma_start(
                            out_d.ap()[chh * 128:(chh + 1) * 128, i0:i0 + 128],
                            res)

            halves = [(i5, g, h) for i5 in range(NI5) for g in range(NCH)
                      for h in range(2)]
            prev = None
            for cur in halves:
                qk(*cur)
                qexp(*cur)
                if prev is not None:
                    pv(*prev)
                    if prev[1] == NCH - 1 and prev[2] == 1:
                        epilogue(prev[0])
                prev = cur
            pv(*prev)
            epilogue(NI5 - 1)

    nc.compile()
    return nc


def _get_nc(has_bq, has_bk, has_bv, neg_g):
    key = (has_bq, has_bk, has_bv, neg_g)
    if key not in _BUILD_CACHE:
        _BUILD_CACHE[key] = _build(*key)
    return _BUILD_CACHE[key]


def kernel(x, w_q, w_k, w_v,
           bn1_scale, bn1_bias, bn1_mean, bn1_var,
           bn2_scale, bn2_bias, bn2_mean, bn2_var,
           bn3_scale, bn3_bias, bn3_mean, bn3_var,
           gamma, _trace=False):
    from concourse.bass_utils import run_bass_kernel_spmd

    x = np.asarray(x, dtype=np.float32)
    gamma_f = float(np.asarray(gamma).reshape(-1)[0])
    bf = ml_dtypes.bfloat16

    def fold(w, s, b, m, v):
        a = np.asarray(s, np.float32) / np.sqrt(np.asarray(v, np.float32) + _EPS)
        return (np.asarray(w, np.float32) * a[:, None],
                np.asarray(b, np.float32) - np.asarray(m, np.float32) * a)

    wqf, bq = fold(w_q, bn1_scale, bn1_bias, bn1_mean, bn1_var)
    wkf, bk = fold(w_k, bn2_scale, bn2_bias, bn2_mean, bn2_var)
    wvf, bv = fold(w_v, bn3_scale, bn3_bias, bn3_mean, bn3_var)
    has_bq = bool(np.any(bq != 0.0))
    has_bk = bool(np.any(bk != 0.0))
    # |gamma| folds into w_v (and bv); sign(gamma) rides the Z column
    ag = abs(gamma_f)
    sg = 1.0 if gamma_f >= 0 else -1.0
    wvf = wvf * ag
    bv = bv * ag
    has_bv = bool(np.any(bv != 0.0))

    nc = _get_nc(has_bq, has_bk, has_bv, sg < 0)

    wqt4 = np.tile(np.ascontiguousarray(wqf.T), (1, 4)).astype(bf)  # [c_in, 4cr]
    wkt4 = np.tile(np.ascontiguousarray(wkf.T), (1, 4)).astype(bf)
    wvt = np.ascontiguousarray(wvf.T).astype(bf)                    # [c_in, c_out]

    in_maps = []
    for core in range(_NCORES):
        b, h = divmod(core, 2)
        # rotate so this core's query half is columns 0:2048 (key order is
        # irrelevant to softmax attention)
        if h == 0:
            xc = x[b]
        else:
            xc = np.concatenate([x[b][:, _HALF:], x[b][:, :_HALF]], axis=1)
        m = {
            "xbf": np.ascontiguousarray(xc.astype(bf)),
            "xqf": np.ascontiguousarray(xc[:, 0:_HALF]),
            "wqt4": wqt4, "wkt4": wkt4, "wvt": wvt,
        }
        if has_bq:
            m["bq4"] = np.ascontiguousarray(np.tile(bq, 4).reshape(4 * _CR, 1))
        if has_bk:
            m["bk4"] = np.ascontiguousarray(np.tile(bk, 4).reshape(4 * _CR, 1))
        if has_bv:
            m["bv"] = np.ascontiguousarray(bv.reshape(1, _C)).astype(bf)
        in_maps.append(m)

    res = run_bass_kernel_spmd(nc, in_maps, core_ids=list(range(_NCORES)),
                               trace=_trace)

    out = np.empty((_B, _C, _N), dtype=np.float32)
    for core in range(_NCORES):
        b, h = divmod(core, 2)
        out[b, :, h * _HALF:(h + 1) * _HALF] = res.results[core]["out"]
    if _trace:
        kernel.last_results = res
    return out
